# revision 1
# baseline (speedup 1.0000x reference)
"""Trainium2 Bass kernel for nn_DecoderTF (masked spectrogram decode + overlap-add).

Computation (per batch m, channel c):
    masked[n, k] = inputs[m, n, k] * est_mask[m, c, n, k]          n in [0,512), k in [0,6000)
    frames[k, l] = sum_n masked[n, k] * W[n, l]                    l in [0,16)
    out[m, c, t] = overlap_and_add(frames, hop=8)                  t in [0,48008)

With L=16 and hop=8, overlap-add reduces to a two-term sum; viewing the output
as out2d[6001, 8]:
    out2d[k, j] = frames[k, j] + frames[k-1, j+8]
                = sum_n masked[n, k]   * W[n, j]
                + sum_n masked[n, k-1] * W[n, j+8]
which is two matmuls (W halves stationary, masked streaming, the second with the
moving operand shifted one column) accumulated into one PSUM tile.  The OLA
costs nothing: the PSUM tile [8, 512] IS a transposed chunk of out2d.

Sharding: data-parallel over M — core m handles inputs[m] / est_mask[m] (no
cross-core communication, W replicated).  Per-core HBM traffic ~37 MB, which is
the roofline for this kernel.
"""

import sys

for _p in ("/opt/trn_rl_repo",):
    if _p not in sys.path:
        sys.path.insert(0, _p)

import numpy as np

import concourse.bass as bass
import concourse.mybir as mybir
from concourse import bacc, masks
from concourse.tile import TileContext
from concourse.bass_utils import run_bass_kernel_spmd

N, L, HOP = 512, 16, 8
K = 6000
C = 2
M = 8
T_OUT = (K - 1) * HOP + L  # 48008
R = K + 1                  # rows of out2d: out2d[k, j] = out[k*8 + j]

F32 = mybir.dt.float32
# float32r streams fp32 bits through the PE's single-pass (reduced internal
# precision) path: 1 cycle/row vs 4 for full fp32.
MM_DT = mybir.dt.float32r

# k-slices (over out2d rows / masked columns), each split into <=512-wide PSUM
# chunks.  1024 = 2 chunks of 512 and a multiple of 128 (transpose sub-tiles);
# narrow slices keep the DMA->compute pipeline fine-grained (small tail).
# Slice order: start with two narrow slices so compute starts on a short
# pipeline fill, and end with narrow slices so the post-stream tail is short.
# Slice (0,1024) runs later: it needs GpSimd memsets for its virtual zero halo
# column and GpSimd spends its first ~5us loading its library.
KSLICES = [
    (1024, 512),
    (1536, 512),
    (0, 1024),
    (2048, 1024),
    (3072, 1024),
    (4096, 1024),
    (5120, 512),
    (5632, 256),
    (5888, R - 5888),  # 113-wide: keeps the post-stream compute tail tiny
]
MAXW = 1025  # widest sbuf tile: 1024 + 1 halo column


def _build_nc(kslices=None, bufs=None, gp_mul=False, m_ring=None, w_late=False):
    kslices = kslices or KSLICES
    bufs = bufs or {}
    # widest tile: slice width + 1 halo column (+1 pad col for odd-width B)
    maxw = max(min(o0 + wks, K) - max(o0 - 1, 0) for o0, wks in kslices) + 2
    nc = bacc.Bacc()
    x = nc.declare_dram_parameter("x", [N, K], F32, isOutput=False)
    mk = nc.declare_dram_parameter("mask", [C, N, K], F32, isOutput=False)
    w = nc.declare_dram_parameter("w", [N, L], MM_DT, isOutput=False)
    out = nc.declare_dram_parameter("out", [C, T_OUT], F32, isOutput=True)

    with TileContext(nc) as tc:
        with (
            tc.tile_pool(name="wp", bufs=1) as wp,
            tc.tile_pool(name="idp", bufs=1) as idp,
            tc.tile_pool(name="xp", bufs=bufs.get("x", 10)) as xp,
            tc.tile_pool(name="mp", bufs=bufs.get("m", 9)) as mp,
            tc.tile_pool(name="mkp", bufs=bufs.get("mk", 12)) as mkp,
            tc.tile_pool(name="frp", bufs=3) as frp,
            tc.tile_pool(name="outp", bufs=4) as outp,
            tc.tile_pool(name="pop", bufs=bufs.get("po", 7), space="PSUM") as pop,
            tc.tile_pool(name="ptp", bufs=bufs.get("pt", 1), space="PSUM") as ptp,
        ):
            # W, layout [p, 16n + l] = W[128n + p, l]; lhsT slices are 8 cols.
            w_t = wp.tile([128, 4 * L], MM_DT)

            def load_w(eng):
                eng.dma_start(
                    out=w_t[:, :].rearrange("p (n l) -> p n l", n=4),
                    in_=w.rearrange("(n p) l -> p n l", p=128),
                )

            if not w_late:
                load_w(nc.sync)
            id_t = idp.tile([8, 8], F32)
            masks.make_identity(nc, id_t[:, :])
            m_eng = nc.scalar if m_ring == "scalar" else nc.sync

            for ks_i, (o0, wks) in enumerate(kslices):
                # tile col j holds masked col (base + j); base = o0-1 so the
                # B-term (shift-by-one) always starts at tile col >= 0.  For
                # ks 0, tile col 0 is a virtual masked[-1] == 0 column.
                base = o0 - 1
                hi = min(o0 + wks, K)
                vt = hi - base               # total tile cols in use
                doff = 0 if o0 > 0 else 1    # col where DMA'd data starts
                dlen = hi - max(base, 0)
                mk_ts = {}
                for n in range(4):
                    x_t = xp.tile([128, maxw], F32, tag="x")
                    if doff:
                        nc.gpsimd.memset(x_t[:, 0:doff], 0.0)
                    nc.sync.dma_start(
                        out=x_t[:, doff : doff + dlen],
                        in_=x[128 * n : 128 * (n + 1), max(base, 0) : hi],
                    )
                    # both channels' masks in one DMA: dram [c, p, k] -> [p, c, k]
                    m_t = mp.tile([128, 2 * maxw], F32, tag="m")
                    if doff:
                        nc.gpsimd.memset(m_t[:, 0:doff], 0.0)
                        nc.gpsimd.memset(m_t[:, maxw : maxw + doff], 0.0)
                    m_eng.dma_start(
                        out=m_t[:, :].rearrange("p (c k) -> p c k", c=2)[
                            :, :, doff : doff + dlen
                        ],
                        in_=mk[:, 128 * n : 128 * (n + 1), max(base, 0) : hi].transpose(
                            [1, 0, 2]
                        ),
                    )
                    if w_late and ks_i == 0 and n == 3:
                        load_w(nc.sync)
                    for c in range(C):
                        mk_t = mkp.tile([128, maxw], MM_DT, tag="mk")
                        # GpSimd is otherwise idle; giving it a quarter of the
                        # multiplies takes DVE off the pipeline critical path
                        eng = (
                            nc.gpsimd
                            if gp_mul and c == 1 and n % 2 == 1
                            else nc.vector
                        )
                        eng.tensor_mul(
                            mk_t[:, 0:vt],
                            x_t[:, 0:vt],
                            m_t[:, c * maxw : c * maxw + vt],
                        )
                        mk_ts[c, n] = mk_t

                chunks = []
                q0 = o0
                while q0 < o0 + wks:
                    chunks.append((q0, min(512, o0 + wks - q0)))
                    q0 += 512

                for c in range(C):
                    fr_t = frp.tile([8, 1024], F32, tag="fr")
                    for q0, wch in chunks:
                        po = pop.tile([8, 512], F32, tag="po")
                        wa = min(wch, K - q0)  # A-term covers [0, wa)
                        # B(n=0) first: it covers the full [0, wch) width
                        ops = [(0, 1), (0, 0)] + [
                            (n, half) for n in range(1, 4) for half in (1, 0)
                        ]
                        for i, (n, half) in enumerate(ops):
                            st, sp = i == 0, i == len(ops) - 1
                            if half == 0:
                                nc.tensor.matmul(
                                    po[0:8, 0:wa],
                                    w_t[:, L * n : L * n + 8],
                                    mk_ts[c, n][:, q0 - base : q0 - base + wa],
                                    start=st, stop=sp,
                                )
                            else:
                                # pad odd widths to even (fp32 PSUM writes are
                                # 8-byte granular); the pad column is never read
                                wb = wch + (wch & 1)
                                nc.tensor.matmul(
                                    po[0:8, 0:wb],
                                    w_t[:, L * n + 8 : L * n + 16],
                                    mk_ts[c, n][:, q0 - 1 - base : q0 - 1 - base + wb],
                                    start=st, stop=sp,
                                )
                        nc.scalar.copy(
                            fr_t[:, q0 - o0 : q0 - o0 + wch], po[0:8, 0:wch]
                        )

                    # transpose [8, 128] slabs -> [128, 8] into one PSUM bank
                    n_sub = (wks + 127) // 128
                    pt = ptp.tile([128, 64], F32, tag="pt")
                    for s in range(n_sub):
                        s0 = 128 * s
                        sw = min(128, wks - s0)
                        nc.tensor.transpose(
                            pt[0:sw, 8 * s : 8 * s + 8],
                            fr_t[:, s0 : s0 + sw],
                            id_t[:, :],
                        )
                    ot = outp.tile([128, 64], F32, tag="ot")
                    nc.scalar.copy(ot[:, 0 : 8 * n_sub], pt[:, 0 : 8 * n_sub])
                    # stream this slice's output rows out now, on the ACT HWDGE
                    # ring so the many tiny strided descriptors don't stall the
                    # input stream's (sync) ring
                    s_full = n_sub if wks % 128 == 0 else n_sub - 1
                    t0 = 8 * o0
                    if s_full:
                        v = out[c, t0 : t0 + 1024 * s_full].rearrange(
                            "(s p j) -> p s j", p=128, j=8
                        )
                        sv = ot[:, 0 : 8 * s_full].rearrange(
                            "p (s j) -> p s j", j=8
                        )
                        nc.scalar.dma_start(out=v, in_=sv)
                    if s_full != n_sub:
                        rem = wks - 128 * s_full  # 113
                        v2 = out[c, t0 + 1024 * s_full : T_OUT].rearrange(
                            "(p j) -> p j", j=8
                        )
                        nc.scalar.dma_start(
                            out=v2, in_=ot[0:rem, 8 * s_full : 8 * n_sub]
                        )
    nc.finalize()
    return nc


_NC_CACHE = None


def _get_nc():
    global _NC_CACHE
    if _NC_CACHE is None:
        _NC_CACHE = _build_nc()
    return _NC_CACHE


def run(inputs, est_mask, W, trace=False):
    """Returns (out [M, C, T_OUT] float32, exec_time_ns or None)."""
    inputs = np.ascontiguousarray(np.asarray(inputs, dtype=np.float32))
    est_mask = np.ascontiguousarray(np.asarray(est_mask, dtype=np.float32))
    W = np.ascontiguousarray(np.asarray(W, dtype=np.float32))
    assert inputs.shape == (M, N, K)
    assert est_mask.shape == (M, C, N, K)
    assert W.shape == (N, L)

    nc = _get_nc()
    in_maps = [
        {"x": inputs[m], "mask": est_mask[m], "w": W} for m in range(M)
    ]
    res = run_bass_kernel_spmd(nc, in_maps, list(range(M)), trace=trace)
    out = np.stack([res.results[m]["out"] for m in range(M)], axis=0)
    return out.astype(np.float32, copy=False), res.exec_time_ns


def kernel(inputs, est_mask, W):
    out, _ = run(inputs, est_mask, W)
    return out

